# revision 3
# baseline (speedup 1.0000x reference)
"""Trainium2 Bass kernel for a custom GRU (B=64, S=512, I=512, H=1024).

Strategy (data-parallel, per the sharding hint):
  - Shard the batch (64) across 8 NeuronCores (8 per core); replicate weights.
  - Phase 1 (per core): x-projections xg = x @ Wg.T + bg for the 3 gates as
    f32r (TF32-like) GEMMs at full PE rate, written to a DRAM scratch buffer
    in a step-major interleaved layout.
  - Phase 2: the sequential recurrence, 512 steps.  Everything lives in a
    transposed "interleaved" layout [128 partitions, 8*m + b] so that all
    elementwise work runs on full 128-partition tiles and the matmul rhs
    slices are contiguous.  U matrices are bf16 (stationary operands, FWL),
    accumulation in fp32 PSUM, gates/state fp32.

Layouts:
  - h (state):  SBUF [128, 64] fp32, element (p, 8*m+b) = h[b, 128*m+p].
  - U tiles:    lhsT per (gate, k-chunk): SBUF [128, 1024] bf16 where
                cols m*128..m*128+128 give the [K=128, M=128] stationary tile.
  - x-proj:     DRAM [512, 128, 192] fp32, (t, p, g*64+8*m+b).
  - out:        DRAM [512, 1024, 8] fp32 (t, h, b_local) per core; assembled
                host-side into [64, 512, 1024] + h_last [1, 64, 1024].
"""

import numpy as np
import ml_dtypes
from contextlib import ExitStack

import concourse.bass as bass
import concourse.mybir as mybir
import concourse.tile as tile
from concourse import bacc
from concourse.bass import ds
from concourse.bass_utils import run_bass_kernel_spmd

B, S, I, H = 64, 512, 512, 1024
NCORES = 8
BL = B // NCORES          # 8 batch per core
MT = H // 128             # 8 m-tiles
KC = H // 128             # 8 recurrence k-chunks
PKC = I // 128            # 4 projection k-chunks
NB = (S * BL) // 512      # 8 projection n-blocks (64 steps each)
UNROLL = 8

F32 = mybir.dt.float32
F32R = mybir.dt.float32r
BF16 = mybir.dt.bfloat16
SIG = mybir.ActivationFunctionType.Sigmoid
TANH = mybir.ActivationFunctionType.Tanh


def _emit(ctx, tc, io, loop_steps, unroll):
    nc = tc.nc
    xT, h0T, WT, UT, bias, out_hT = (
        io["xT"], io["h0T"], io["WT"], io["UT"], io["bias"], io["out_hT"])

    const_pool = ctx.enter_context(tc.tile_pool(name="const", bufs=1))
    xt_pool = ctx.enter_context(tc.tile_pool(name="xt", bufs=1))
    w_pool = ctx.enter_context(tc.tile_pool(name="w", bufs=1))
    u_pool = ctx.enter_context(tc.tile_pool(name="u", bufs=1))
    state_pool = ctx.enter_context(tc.tile_pool(name="state", bufs=1))
    stage_pool = ctx.enter_context(tc.tile_pool(name="stage", bufs=3))
    ppsum_pool = ctx.enter_context(tc.tile_pool(name="ppsum", bufs=2, space="PSUM"))
    xz_pool = ctx.enter_context(tc.tile_pool(name="xz", bufs=3))
    zrp_pool = ctx.enter_context(tc.tile_pool(name="zrp", bufs=2, space="PSUM"))
    hp_pool = ctx.enter_context(tc.tile_pool(name="hp", bufs=2, space="PSUM"))
    ew_pool = ctx.enter_context(tc.tile_pool(name="ew", bufs=2))
    dram_pool = ctx.enter_context(tc.tile_pool(name="dram", bufs=1, space="DRAM"))

    # DRAM scratch for the projected inputs, step-major interleaved.
    xp = dram_pool.tile([S, 128, 3 * 64], F32)

    # ---- load constants -------------------------------------------------
    xt_tiles = []
    for k in range(PKC):
        t = xt_pool.tile([128, S * BL], F32R, tag=f"xt{k}", name=f"xt{k}")
        nc.sync.dma_start(t[:].rearrange("p (s b) -> p s b", b=BL),
                          xT[k * 128:(k + 1) * 128, :, :])
        xt_tiles.append(t)

    w_tiles = {}
    for g in range(3):
        for k in range(PKC):
            t = w_pool.tile([128, H], F32R, tag=f"w{g}_{k}", name=f"w{g}_{k}")
            nc.sync.dma_start(t[:], WT[g, k * 128:(k + 1) * 128, :])
            w_tiles[(g, k)] = t

    u_tiles = {}
    for g in range(3):
        for k in range(KC):
            t = u_pool.tile([128, H], BF16, tag=f"u{g}_{k}", name=f"u{g}_{k}")
            nc.sync.dma_start(t[:], UT[g, k * 128:(k + 1) * 128, :])
            u_tiles[(g, k)] = t

    bias_sb = []
    for g in range(3):
        t = const_pool.tile([128, MT], F32, tag=f"b{g}", name=f"bias{g}")
        nc.sync.dma_start(t[:].rearrange("p m -> p m"),
                          bias[g, :].rearrange("(m p) -> p m", p=128))
        bias_sb.append(t)

    # ---- phase 1: projections ------------------------------------------
    for g in range(3):
        for m in range(MT):
            bsl = bias_sb[g][:, m:m + 1]
            for n in range(NB):
                ps = ppsum_pool.tile([128, 512], F32)
                for k in range(PKC):
                    nc.tensor.matmul(
                        ps[:],
                        w_tiles[(g, k)][:, m * 128:(m + 1) * 128],
                        xt_tiles[k][:, n * 512:(n + 1) * 512],
                        start=(k == 0), stop=(k == PKC - 1))
                st = stage_pool.tile([128, 512], F32, tag="pstage")
                nc.vector.tensor_scalar_add(st[:], ps[:], bsl)
                nc.sync.dma_start(
                    xp[n * 64:(n + 1) * 64, :, g * 64 + 8 * m: g * 64 + 8 * m + 8]
                    .rearrange("s p b -> p s b"),
                    st[:].rearrange("p (s b) -> p s b", b=BL))

    # ---- phase 2: recurrence -------------------------------------------
    h_t = [state_pool.tile([128, MT * BL], F32, tag=f"h{i}", name=f"h{i}") for i in range(2)]
    hbf = state_pool.tile([128, MT * BL], BF16, tag="hbf")
    nc.sync.dma_start(h_t[0][:].rearrange("p (m b) -> p m b", b=BL),
                      h0T.rearrange("(m p) b -> p m b", p=128))
    nc.vector.tensor_copy(hbf[:], h_t[0][:])

    def step(t_ap, h_prev, h_next):
        xz_t = xz_pool.tile([128, 192], F32, tag="xz")
        nc.sync.dma_start(xz_t[:], t_ap)

        psum_zr = zrp_pool.tile([128, 128], F32, tag="psum_zr")
        for m in range(MT):
            for gi, cb in ((0, 0), (1, 64)):
                for k in range(KC):
                    nc.tensor.matmul(
                        psum_zr[:, cb + 8 * m: cb + 8 * m + 8],
                        u_tiles[(gi, k)][:, m * 128:(m + 1) * 128],
                        hbf[:, 8 * k:8 * k + 8],
                        start=(k == 0), stop=(k == KC - 1))
        tmp_zr = ew_pool.tile([128, 128], F32, tag="tmp_zr")
        nc.vector.tensor_add(tmp_zr[:], psum_zr[:], xz_t[:, 0:128])
        zr_s = ew_pool.tile([128, 128], F32, tag="zr_s")
        nc.scalar.activation(zr_s[:], tmp_zr[:], SIG)
        rh_bf = ew_pool.tile([128, 64], BF16, tag="rh")
        nc.vector.tensor_mul(rh_bf[:], zr_s[:, 64:128], h_prev[:])

        psum_h = hp_pool.tile([128, 64], F32, tag="psum_h")
        for m in range(MT):
            for k in range(KC):
                nc.tensor.matmul(
                    psum_h[:, 8 * m:8 * m + 8],
                    u_tiles[(2, k)][:, m * 128:(m + 1) * 128],
                    rh_bf[:, 8 * k:8 * k + 8],
                    start=(k == 0), stop=(k == KC - 1))
        tmp_h = ew_pool.tile([128, 64], F32, tag="tmp_h")
        nc.vector.tensor_add(tmp_h[:], psum_h[:], xz_t[:, 128:192])
        hh = ew_pool.tile([128, 64], F32, tag="hh")
        nc.scalar.activation(hh[:], tmp_h[:], TANH)
        dd = ew_pool.tile([128, 64], F32, tag="dd")
        nc.vector.tensor_sub(dd[:], hh[:], h_prev[:])
        zd = ew_pool.tile([128, 64], F32, tag="zd")
        nc.vector.tensor_mul(zd[:], zr_s[:, 0:64], dd[:])
        nc.vector.tensor_add(h_next[:], h_prev[:], zd[:])
        nc.vector.tensor_copy(hbf[:], h_next[:])
        return h_next

    def emit_step(t_idx, j):
        h_prev = h_t[j % 2]
        h_next = h_t[(j + 1) % 2]
        step(xp[ds(t_idx, 1), :, :], h_prev, h_next)
        nc.sync.dma_start(
            out_hT[ds(t_idx, 1), :, :].rearrange("t (m p) b -> t p m b", p=128),
            h_next[:].rearrange("p (m b) -> p m b", b=BL))

    if loop_steps >= unroll:
        with tc.For_i(0, loop_steps - loop_steps % unroll, unroll,
                      hint_engines=(mybir.EngineType.PE,)) as iv:
            for j in range(unroll):
                emit_step(iv + j, j)
    for j in range(loop_steps % unroll):
        emit_step(loop_steps - loop_steps % unroll + j, j)


def build(loop_steps=S, unroll=UNROLL):
    nc = bacc.Bacc("TRN2", target_bir_lowering=False, debug=False,
                   num_devices=NCORES)
    io = {
        "xT": nc.dram_tensor("xT", [I, S, BL], F32R, kind="ExternalInput").ap(),
        "h0T": nc.dram_tensor("h0T", [H, BL], F32, kind="ExternalInput").ap(),
        "WT": nc.dram_tensor("WT", [3, I, H], F32R, kind="ExternalInput").ap(),
        "UT": nc.dram_tensor("UT", [3, H, H], BF16, kind="ExternalInput").ap(),
        "bias": nc.dram_tensor("bias", [3, H], F32, kind="ExternalInput").ap(),
        "out_hT": nc.dram_tensor("out_hT", [S, H, BL], F32,
                                 kind="ExternalOutput").ap(),
    }
    with tile.TileContext(nc) as tc:
        with ExitStack() as ctx:
            _emit(ctx, tc, io, loop_steps, unroll)
    nc.compile()
    return nc


def make_in_maps(x, h_0, Wz, bz, Uz, Wr, br, Ur, Wh, bh, Uh):
    x = np.ascontiguousarray(np.asarray(x, dtype=np.float32))
    h_0 = np.asarray(h_0, dtype=np.float32)
    WT = np.ascontiguousarray(
        np.stack([np.asarray(w, np.float32).T for w in (Wz, Wr, Wh)]))
    UT = np.ascontiguousarray(
        np.stack([np.asarray(u, np.float32).T for u in (Uz, Ur, Uh)])
    ).astype(ml_dtypes.bfloat16)
    bias = np.ascontiguousarray(
        np.stack([np.asarray(b, np.float32) for b in (bz, br, bh)]))
    in_maps = []
    for c in range(NCORES):
        xT_c = np.ascontiguousarray(
            x[c * BL:(c + 1) * BL].transpose(2, 1, 0))          # [I, S, BL]
        h0T_c = np.ascontiguousarray(
            np.asarray(h_0, np.float32)[0, c * BL:(c + 1) * BL].T)  # [H, BL]
        in_maps.append({"xT": xT_c, "h0T": h0T_c, "WT": WT, "UT": UT,
                        "bias": bias})
    return in_maps


def assemble(results):
    # per-core out_hT [S, H, BL] -> hidden_seq [B, S, H]
    hidden = np.empty((B, S, H), np.float32)
    for c in range(NCORES):
        # [S, H, BL] -> [BL, S, H]
        hidden[c * BL:(c + 1) * BL] = results[c]["out_hT"].transpose(2, 0, 1)
    h_last = np.ascontiguousarray(hidden[:, -1, :])[None]
    return hidden, h_last


_cached_nc = None


def kernel(x, h_0, Wz, bz, Uz, Wr, br, Ur, Wh, bh, Uh, _trace=False):
    global _cached_nc
    if _cached_nc is None:
        _cached_nc = build()
    in_maps = make_in_maps(x, h_0, Wz, bz, Uz, Wr, br, Ur, Wh, bh, Uh)
    res = run_bass_kernel_spmd(_cached_nc, in_maps,
                               core_ids=list(range(NCORES)), trace=_trace)
    out = assemble(res.results)
    if _trace:
        return out, res
    return out


# revision 7
# speedup vs baseline: 1.3443x; 1.3443x over previous
"""Trainium2 Bass kernel for a custom GRU (B=64, S=512, I=512, H=1024).

Strategy (data-parallel, per the sharding hint):
  - Shard the batch (64) across 8 NeuronCores (8 per core); replicate weights.
  - Phase 1 (per core): x-projections xg = x @ Wg.T + bg for the 3 gates as
    f32r (TF32-like) GEMMs at full PE rate, written to a DRAM scratch buffer
    in a step-major interleaved layout.
  - Phase 2: the sequential recurrence, 512 steps.  Everything lives in a
    transposed "interleaved" layout [128 partitions, 8*m + b] so that all
    elementwise work runs on full 128-partition tiles and the matmul rhs
    slices are contiguous.  U matrices are bf16 (stationary operands, FWL),
    accumulation in fp32 PSUM, gates/state fp32.

Layouts:
  - h (state):  SBUF [128, 64] fp32, element (p, 8*m+b) = h[b, 128*m+p].
  - U tiles:    lhsT per (gate, k-chunk): SBUF [128, 1024] bf16 where
                cols m*128..m*128+128 give the [K=128, M=128] stationary tile.
  - x-proj:     DRAM [512, 128, 192] fp32, (t, p, g*64+8*m+b).
  - out:        DRAM [512, 1024, 8] fp32 (t, h, b_local) per core; assembled
                host-side into [64, 512, 1024] + h_last [1, 64, 1024].
"""

import numpy as np
import ml_dtypes
from contextlib import ExitStack

import concourse.bass as bass
import concourse.mybir as mybir
import concourse.tile as tile
from concourse import bacc
from concourse.bass import ds
from concourse.bass_utils import run_bass_kernel_spmd

B, S, I, H = 64, 512, 512, 1024
NCORES = 8
BL = B // NCORES          # 8 batch per core
MT = H // 128             # 8 m-tiles
KC = H // 128             # 8 recurrence k-chunks
PKC = I // 128            # 4 projection k-chunks
NB = (S * BL) // 512      # 8 projection n-blocks (64 steps each)
UNROLL = 16

F32 = mybir.dt.float32
F32R = mybir.dt.float32r
BF16 = mybir.dt.bfloat16
SIG = mybir.ActivationFunctionType.Sigmoid
TANH = mybir.ActivationFunctionType.Tanh


def _emit(ctx, tc, io, loop_steps, unroll):
    nc = tc.nc
    xT, h0T, WT, UT, bias, out_hT = (
        io["xT"], io["h0T"], io["WT"], io["UT"], io["bias"], io["out_hT"])

    const_pool = ctx.enter_context(tc.tile_pool(name="const", bufs=1))
    xt_pool = ctx.enter_context(tc.tile_pool(name="xt", bufs=1))
    w_pool = ctx.enter_context(tc.tile_pool(name="w", bufs=1))
    u_pool = ctx.enter_context(tc.tile_pool(name="u", bufs=1))
    state_pool = ctx.enter_context(tc.tile_pool(name="state", bufs=1))
    stage_pool = ctx.enter_context(tc.tile_pool(name="stage", bufs=3))
    psum_pool = ctx.enter_context(tc.tile_pool(name="psum", bufs=2, space="PSUM"))
    xz_pool = ctx.enter_context(tc.tile_pool(name="xz", bufs=3))
    ew_pool = ctx.enter_context(tc.tile_pool(name="ew", bufs=2))
    dram_pool = ctx.enter_context(tc.tile_pool(name="dram", bufs=1, space="DRAM"))

    # DRAM scratch for the projected inputs: [gate, m, p, t, b] so that the
    # projection evacuation writes 2KB-contiguous per-partition runs and the
    # per-step read is 3 small strided DMAs.
    xp = dram_pool.tile([3, MT, 128, S, BL], F32)

    # ---- load constants -------------------------------------------------
    xt_tiles = []
    for k in range(PKC):
        t = xt_pool.tile([128, S * BL], F32R, tag=f"xt{k}", name=f"xt{k}")
        nc.sync.dma_start(t[:].rearrange("p (s b) -> p s b", b=BL),
                          xT[k * 128:(k + 1) * 128, :, :])
        xt_tiles.append(t)

    w_tiles = {}
    for g in range(3):
        for k in range(PKC):
            t = w_pool.tile([128, H], F32R, tag=f"w{g}_{k}", name=f"w{g}_{k}")
            nc.sync.dma_start(t[:], WT[g, k * 128:(k + 1) * 128, :])
            w_tiles[(g, k)] = t

    u_tiles = {}
    for g in range(3):
        for k in range(KC):
            t = u_pool.tile([128, H], BF16, tag=f"u{g}_{k}", name=f"u{g}_{k}")
            nc.sync.dma_start(t[:], UT[g, k * 128:(k + 1) * 128, :])
            u_tiles[(g, k)] = t

    bias_sb = []
    for g in range(3):
        t = const_pool.tile([128, MT], F32, tag=f"b{g}", name=f"bias{g}")
        nc.sync.dma_start(t[:].rearrange("p m -> p m"),
                          bias[g, :].rearrange("(m p) -> p m", p=128))
        bias_sb.append(t)

    PTAGS = ["p_r", "p_z", "p_hlo", "p_hhi"]

    # ---- phase 1: projections ------------------------------------------
    for g in range(3):
        for m in range(MT):
            bsl = bias_sb[g][:, m:m + 1]
            for n in range(NB):
                ps = psum_pool.tile([128, 512], F32, tag=PTAGS[n % 4],
                                    name=f"pp{g}_{m}_{n}")
                for k in range(PKC):
                    nc.tensor.matmul(
                        ps[:],
                        w_tiles[(g, k)][:, m * 128:(m + 1) * 128],
                        xt_tiles[k][:, n * 512:(n + 1) * 512],
                        start=(k == 0), stop=(k == PKC - 1))
                st = stage_pool.tile([128, 512], F32, tag="pstage",
                                     name=f"st{g}_{m}_{n}")
                nc.vector.tensor_scalar_add(st[:], ps[:], bsl)
                nc.sync.dma_start(
                    xp[g, m, :, n * 64:(n + 1) * 64, :],
                    st[:].rearrange("p (s b) -> p s b", b=BL))

    # ---- phase boundary: keep the loop's SBUF reuse ordered after the
    # projection phase's last reads.
    tc.strict_bb_all_engine_barrier()

    # ---- phase 2: recurrence -------------------------------------------
    h_t = [state_pool.tile([128, MT * BL], F32, tag=f"h{i}", name=f"h{i}")
           for i in range(2)]
    hbf = state_pool.tile([128, MT * BL], BF16, tag="hbf", name="hbf")
    nc.sync.dma_start(h_t[0][:].rearrange("p (m b) -> p m b", b=BL),
                      h0T.rearrange("(m p) b -> p m b", p=128))
    nc.vector.tensor_copy(hbf[:], h_t[0][:])

    def step(t_idx, h_prev, h_next):
        # per-step inputs: 3 strided DMAs (one per gate) into [128, 192]
        xz_t = xz_pool.tile([128, 192], F32, tag="xz", name="xz")
        for g in range(3):
            nc.sync.dma_start(
                xz_t[:, g * 64:(g + 1) * 64],
                xp[g, :, :, ds(t_idx, 1), :].rearrange("m p t b -> p m (t b)"))

        # r-gate matmuls first (own bank) so its sigmoid chain hides under z.
        psum_r = psum_pool.tile([128, 64], F32, tag="p_r", name="psum_r")
        for m in range(MT):
            for k in range(KC):
                nc.tensor.matmul(
                    psum_r[:, 8 * m:8 * m + 8],
                    u_tiles[(1, k)][:, m * 128:(m + 1) * 128],
                    hbf[:, 8 * k:8 * k + 8],
                    start=(k == 0), stop=(k == KC - 1))
        tmp_r = ew_pool.tile([128, 64], F32, tag="tmp_r", name="tmp_r")
        nc.vector.tensor_add(tmp_r[:], psum_r[:], xz_t[:, 64:128])
        s_r = ew_pool.tile([128, 64], F32, tag="s_r", name="s_r")
        nc.scalar.activation(s_r[:], tmp_r[:], SIG)
        rh_bf = ew_pool.tile([128, 64], BF16, tag="rh", name="rh_bf")
        nc.vector.tensor_mul(rh_bf[:], s_r[:], h_prev[:])

        psum_z = psum_pool.tile([128, 64], F32, tag="p_z", name="psum_z")
        for m in range(MT):
            for k in range(KC):
                nc.tensor.matmul(
                    psum_z[:, 8 * m:8 * m + 8],
                    u_tiles[(0, k)][:, m * 128:(m + 1) * 128],
                    hbf[:, 8 * k:8 * k + 8],
                    start=(k == 0), stop=(k == KC - 1))
        tmp_z = ew_pool.tile([128, 64], F32, tag="tmp_z", name="tmp_z")
        nc.vector.tensor_add(tmp_z[:], psum_z[:], xz_t[:, 0:64])
        s_z = ew_pool.tile([128, 64], F32, tag="s_z", name="s_z")
        nc.scalar.activation(s_z[:], tmp_z[:], SIG)

        # h-hat matmuls, split in half (two banks) so the lo tail overlaps
        # the hi matmuls.
        psum_h = [psum_pool.tile([128, 32], F32, tag=PTAGS[2 + hh],
                                 name=f"psum_h{hh}") for hh in range(2)]
        for hh in range(2):
            for m in range(4 * hh, 4 * hh + 4):
                for k in range(KC):
                    nc.tensor.matmul(
                        psum_h[hh][:, 8 * (m - 4 * hh): 8 * (m - 4 * hh) + 8],
                        u_tiles[(2, k)][:, m * 128:(m + 1) * 128],
                        rh_bf[:, 8 * k:8 * k + 8],
                        start=(k == 0), stop=(k == KC - 1))
        for hh in range(2):
            sl = slice(32 * hh, 32 * hh + 32)
            tmp_h = ew_pool.tile([128, 32], F32, tag=f"tmp_h{hh}",
                                 name=f"tmp_h{hh}")
            nc.vector.tensor_add(tmp_h[:], psum_h[hh][:],
                                 xz_t[:, 128 + 32 * hh:160 + 32 * hh])
            hh_s = ew_pool.tile([128, 32], F32, tag=f"hh_s{hh}",
                                name=f"hh_s{hh}")
            nc.scalar.activation(hh_s[:], tmp_h[:], TANH)
            dd = ew_pool.tile([128, 32], F32, tag=f"dd{hh}", name=f"dd{hh}")
            nc.vector.tensor_sub(dd[:], hh_s[:], h_prev[:, sl])
            zd = ew_pool.tile([128, 32], F32, tag=f"zd{hh}", name=f"zd{hh}")
            nc.vector.tensor_mul(zd[:], s_z[:, sl], dd[:])
            nc.vector.tensor_add(h_next[:, sl], h_prev[:, sl], zd[:])
            nc.vector.tensor_copy(hbf[:, sl], h_next[:, sl])

    def emit_step(t_idx, j):
        h_prev = h_t[j % 2]
        h_next = h_t[(j + 1) % 2]
        step(t_idx, h_prev, h_next)
        nc.sync.dma_start(
            out_hT[ds(t_idx, 1), :, :].rearrange("t (m p) b -> t p m b", p=128),
            h_next[:].rearrange("p (m b) -> p m b", b=BL))

    if loop_steps >= unroll:
        with tc.For_i(0, loop_steps - loop_steps % unroll, unroll,
                      hint_engines=(mybir.EngineType.PE,)) as iv:
            for j in range(unroll):
                emit_step(iv + j, j)
    for j in range(loop_steps % unroll):
        emit_step(loop_steps - loop_steps % unroll + j, j)


def build(loop_steps=S, unroll=UNROLL):
    nc = bacc.Bacc("TRN2", target_bir_lowering=False, debug=False,
                   num_devices=NCORES)
    io = {
        "xT": nc.dram_tensor("xT", [I, S, BL], F32R, kind="ExternalInput").ap(),
        "h0T": nc.dram_tensor("h0T", [H, BL], F32, kind="ExternalInput").ap(),
        "WT": nc.dram_tensor("WT", [3, I, H], F32R, kind="ExternalInput").ap(),
        "UT": nc.dram_tensor("UT", [3, H, H], BF16, kind="ExternalInput").ap(),
        "bias": nc.dram_tensor("bias", [3, H], F32, kind="ExternalInput").ap(),
        "out_hT": nc.dram_tensor("out_hT", [S, H, BL], F32,
                                 kind="ExternalOutput").ap(),
    }
    with tile.TileContext(nc) as tc:
        with ExitStack() as ctx:
            _emit(ctx, tc, io, loop_steps, unroll)
    nc.compile()
    return nc


def make_in_maps(x, h_0, Wz, bz, Uz, Wr, br, Ur, Wh, bh, Uh):
    x = np.ascontiguousarray(np.asarray(x, dtype=np.float32))
    h_0 = np.asarray(h_0, dtype=np.float32)
    WT = np.ascontiguousarray(
        np.stack([np.asarray(w, np.float32).T for w in (Wz, Wr, Wh)]))
    UT = np.ascontiguousarray(
        np.stack([np.asarray(u, np.float32).T for u in (Uz, Ur, Uh)])
    ).astype(ml_dtypes.bfloat16)
    bias = np.ascontiguousarray(
        np.stack([np.asarray(b, np.float32) for b in (bz, br, bh)]))
    in_maps = []
    for c in range(NCORES):
        xT_c = np.ascontiguousarray(
            x[c * BL:(c + 1) * BL].transpose(2, 1, 0))          # [I, S, BL]
        h0T_c = np.ascontiguousarray(
            np.asarray(h_0, np.float32)[0, c * BL:(c + 1) * BL].T)  # [H, BL]
        in_maps.append({"xT": xT_c, "h0T": h0T_c, "WT": WT, "UT": UT,
                        "bias": bias})
    return in_maps


def assemble(results):
    # per-core out_hT [S, H, BL] -> hidden_seq [B, S, H]
    hidden = np.empty((B, S, H), np.float32)
    for c in range(NCORES):
        # [S, H, BL] -> [BL, S, H]
        hidden[c * BL:(c + 1) * BL] = results[c]["out_hT"].transpose(2, 0, 1)
    h_last = np.ascontiguousarray(hidden[:, -1, :])[None]
    return hidden, h_last


_cached_nc = None


def kernel(x, h_0, Wz, bz, Uz, Wr, br, Ur, Wh, bh, Uh, _trace=False):
    global _cached_nc
    if _cached_nc is None:
        _cached_nc = build()
    in_maps = make_in_maps(x, h_0, Wz, bz, Uz, Wr, br, Ur, Wh, bh, Uh)
    res = run_bass_kernel_spmd(_cached_nc, in_maps,
                               core_ids=list(range(NCORES)), trace=_trace)
    out = assemble(res.results)
    if _trace:
        return out, res
    return out


# revision 11
# speedup vs baseline: 1.3726x; 1.0211x over previous
"""Trainium2 Bass kernel for a custom GRU (B=64, S=512, I=512, H=1024).

Strategy (data-parallel, per the sharding hint):
  - Shard the batch (64) across 8 NeuronCores (8 per core); replicate weights.
  - Phase 1 (per core): x-projections xg = x @ Wg.T + bg for the 3 gates as
    f32r (TF32-like) GEMMs at full PE rate, written to a DRAM scratch buffer
    in a step-major interleaved layout.
  - Phase 2: the sequential recurrence, 512 steps.  Everything lives in a
    transposed "interleaved" layout [128 partitions, 8*m + b] so that all
    elementwise work runs on full 128-partition tiles and the matmul rhs
    slices are contiguous.  U matrices are bf16 (stationary operands, FWL),
    accumulation in fp32 PSUM, gates/state fp32.

Layouts:
  - h (state):  SBUF [128, 64] fp32, element (p, 8*m+b) = h[b, 128*m+p].
  - U tiles:    lhsT per (gate, k-chunk): SBUF [128, 1024] bf16 where
                cols m*128..m*128+128 give the [K=128, M=128] stationary tile.
  - x-proj:     DRAM [512, 128, 192] fp32, (t, p, g*64+8*m+b).
  - out:        DRAM [512, 1024, 8] fp32 (t, h, b_local) per core; assembled
                host-side into [64, 512, 1024] + h_last [1, 64, 1024].
"""

import numpy as np
import ml_dtypes
from contextlib import ExitStack

import concourse.bass as bass
import concourse.mybir as mybir
import concourse.tile as tile
from concourse import bacc
from concourse.bass import ds
from concourse.bass_utils import run_bass_kernel_spmd

B, S, I, H = 64, 512, 512, 1024
NCORES = 8
BL = B // NCORES          # 8 batch per core
MT = H // 128             # 8 m-tiles
KC = H // 128             # 8 recurrence k-chunks
PKC = I // 128            # 4 projection k-chunks
NB = (S * BL) // 512      # 8 projection n-blocks (64 steps each)
UNROLL = 32

F32 = mybir.dt.float32
F32R = mybir.dt.float32r
BF16 = mybir.dt.bfloat16
SIG = mybir.ActivationFunctionType.Sigmoid
TANH = mybir.ActivationFunctionType.Tanh


def _emit(ctx, tc, io, loop_steps, unroll):
    nc = tc.nc
    xT, h0T, WT, UT, bias, out_hT = (
        io["xT"], io["h0T"], io["WT"], io["UT"], io["bias"], io["out_hT"])

    const_pool = ctx.enter_context(tc.tile_pool(name="const", bufs=1))
    xt_pool = ctx.enter_context(tc.tile_pool(name="xt", bufs=1))
    w_pool = ctx.enter_context(tc.tile_pool(name="w", bufs=1))
    u_pool = ctx.enter_context(tc.tile_pool(name="u", bufs=1))
    state_pool = ctx.enter_context(tc.tile_pool(name="state", bufs=1))
    stage_pool = ctx.enter_context(tc.tile_pool(name="stage", bufs=3))
    psum_pool = ctx.enter_context(tc.tile_pool(name="psum", bufs=2, space="PSUM"))
    xz_pool = ctx.enter_context(tc.tile_pool(name="xz", bufs=3))
    ew_pool = ctx.enter_context(tc.tile_pool(name="ew", bufs=2))
    dram_pool = ctx.enter_context(tc.tile_pool(name="dram", bufs=1, space="DRAM"))

    # DRAM scratch for the projected inputs: [gate, m, p, t, b] so that the
    # projection evacuation writes 2KB-contiguous per-partition runs and the
    # per-step read is 3 small strided DMAs.
    xp = dram_pool.tile([3, MT, 128, S, BL], F32)

    # ---- load constants -------------------------------------------------
    xt_tiles = []
    for k in range(PKC):
        t = xt_pool.tile([128, S * BL], F32R, tag=f"xt{k}", name=f"xt{k}")
        nc.sync.dma_start(t[:].rearrange("p (s b) -> p s b", b=BL),
                          xT[k * 128:(k + 1) * 128, :, :])
        xt_tiles.append(t)

    w_tiles = {}
    for g in range(3):
        for k in range(PKC):
            t = w_pool.tile([128, H], F32R, tag=f"w{g}_{k}", name=f"w{g}_{k}")
            nc.sync.dma_start(t[:], WT[g, k * 128:(k + 1) * 128, :])
            w_tiles[(g, k)] = t

    u_tiles = {}
    for g in range(3):
        for k in range(KC):
            t = u_pool.tile([128, H], BF16, tag=f"u{g}_{k}", name=f"u{g}_{k}")
            nc.sync.dma_start(t[:], UT[g, k * 128:(k + 1) * 128, :])
            u_tiles[(g, k)] = t

    bias_sb = []
    for g in range(3):
        t = const_pool.tile([128, MT], F32, tag=f"b{g}", name=f"bias{g}")
        nc.sync.dma_start(t[:].rearrange("p m -> p m"),
                          bias[g, :].rearrange("(m p) -> p m", p=128))
        bias_sb.append(t)

    PTAGS = ["p_ra", "p_rb", "p_hlo", "p_hhi"]

    # ---- phase 1: projections ------------------------------------------
    for g in range(3):
        for m in range(MT):
            bsl = bias_sb[g][:, m:m + 1]
            for n in range(NB):
                ps = psum_pool.tile([128, 512], F32, tag=PTAGS[n % 2],
                                    name=f"pp{g}_{m}_{n}")
                for k in range(PKC):
                    nc.tensor.matmul(
                        ps[:],
                        w_tiles[(g, k)][:, m * 128:(m + 1) * 128],
                        xt_tiles[k][:, n * 512:(n + 1) * 512],
                        start=(k == 0), stop=(k == PKC - 1))
                st = stage_pool.tile([128, 512], F32, tag="pstage",
                                     name=f"st{g}_{m}_{n}")
                nc.vector.tensor_scalar_add(st[:], ps[:], bsl)
                nc.sync.dma_start(
                    xp[g, m, :, n * 64:(n + 1) * 64, :],
                    st[:].rearrange("p (s b) -> p s b", b=BL))

    # ---- phase boundary: keep the loop's SBUF reuse ordered after the
    # projection phase's last reads.
    tc.strict_bb_all_engine_barrier()

    # ---- phase 2: recurrence -------------------------------------------
    h_t = [state_pool.tile([128, MT * BL], F32, tag=f"h{i}", name=f"h{i}")
           for i in range(2)]
    hbf = state_pool.tile([128, MT * BL], BF16, tag="hbf", name="hbf")
    nc.sync.dma_start(h_t[0][:].rearrange("p (m b) -> p m b", b=BL),
                      h0T.rearrange("(m p) b -> p m b", p=128))
    nc.vector.tensor_copy(hbf[:], h_t[0][:])

    def step(t_idx, h_prev, h_next):
        # per-step inputs: 3 strided DMAs (one per gate) into [128, 192]
        xz_t = xz_pool.tile([128, 192], F32, tag="xz", name="xz")
        for g in range(3):
            nc.sync.dma_start(
                xz_t[:, g * 64:(g + 1) * 64],
                xp[g, :, :, ds(t_idx, 1), :].rearrange("m p t b -> p m (t b)"))

        # r-gate matmuls first, k-split across two banks so the first half
        # only depends on the low half of hbf (overlaps the previous step's
        # high-half tail).
        psum_ra = psum_pool.tile([128, 64], F32, tag="p_ra", name="psum_ra")
        psum_rb = psum_pool.tile([128, 64], F32, tag="p_rb", name="psum_rb")
        for half, prr in ((0, psum_ra), (1, psum_rb)):
            for m in range(MT):
                for k in range(4 * half, 4 * half + 4):
                    nc.tensor.matmul(
                        prr[:, 8 * m:8 * m + 8],
                        u_tiles[(1, k)][:, m * 128:(m + 1) * 128],
                        hbf[:, 8 * k:8 * k + 8],
                        start=(k == 4 * half), stop=(k == 4 * half + 3))
        tmp_r = ew_pool.tile([128, 64], F32, tag="tmp_r", name="tmp_r")
        nc.vector.tensor_add(tmp_r[:], psum_ra[:], xz_t[:, 64:128])
        tmp_r2 = ew_pool.tile([128, 64], F32, tag="tmp_r2", name="tmp_r2")
        nc.vector.tensor_add(tmp_r2[:], tmp_r[:], psum_rb[:])
        s_r = ew_pool.tile([128, 64], F32, tag="s_r", name="s_r")
        nc.scalar.activation(s_r[:], tmp_r2[:], SIG)
        rh_bf = ew_pool.tile([128, 64], BF16, tag="rh", name="rh_bf")
        nc.vector.tensor_mul(rh_bf[:], s_r[:], h_prev[:])

        psum_z = psum_pool.tile([128, 64], F32, tag="p_z", name="psum_z",
                                bufs=1)
        for m in range(MT):
            for k in range(KC):
                nc.tensor.matmul(
                    psum_z[:, 8 * m:8 * m + 8],
                    u_tiles[(0, k)][:, m * 128:(m + 1) * 128],
                    hbf[:, 8 * k:8 * k + 8],
                    start=(k == 0), stop=(k == KC - 1))

        # h-hat matmuls, split in half (two banks) so the lo tail overlaps
        # the hi matmuls.
        psum_h = [psum_pool.tile([128, 32], F32, tag=PTAGS[2 + hh],
                                 name=f"psum_h{hh}", bufs=1) for hh in range(2)]
        for hh in range(2):
            for m in range(4 * hh, 4 * hh + 4):
                for k in range(KC):
                    nc.tensor.matmul(
                        psum_h[hh][:, 8 * (m - 4 * hh): 8 * (m - 4 * hh) + 8],
                        u_tiles[(2, k)][:, m * 128:(m + 1) * 128],
                        rh_bf[:, 8 * k:8 * k + 8],
                        start=(k == 0), stop=(k == KC - 1))
        # z's elementwise chain is only needed by the tail; emit it after the
        # h-hat matmuls so the DVE FIFO doesn't delay rh behind the z PSUM.
        tmp_z = ew_pool.tile([128, 64], F32, tag="tmp_z", name="tmp_z")
        nc.vector.tensor_add(tmp_z[:], psum_z[:], xz_t[:, 0:64])
        s_z = ew_pool.tile([128, 64], F32, tag="s_z", name="s_z")
        nc.scalar.activation(s_z[:], tmp_z[:], SIG)
        for hh in range(2):
            sl = slice(32 * hh, 32 * hh + 32)
            tmp_h = ew_pool.tile([128, 32], F32, tag=f"tmp_h{hh}",
                                 name=f"tmp_h{hh}")
            nc.vector.tensor_add(tmp_h[:], psum_h[hh][:],
                                 xz_t[:, 128 + 32 * hh:160 + 32 * hh])
            hh_s = ew_pool.tile([128, 32], F32, tag=f"hh_s{hh}",
                                name=f"hh_s{hh}")
            nc.scalar.activation(hh_s[:], tmp_h[:], TANH)
            dd = ew_pool.tile([128, 32], F32, tag=f"dd{hh}", name=f"dd{hh}")
            nc.vector.tensor_sub(dd[:], hh_s[:], h_prev[:, sl])
            zd = ew_pool.tile([128, 32], F32, tag=f"zd{hh}", name=f"zd{hh}")
            nc.vector.tensor_mul(zd[:], s_z[:, sl], dd[:])
            nc.vector.tensor_add(h_next[:, sl], h_prev[:, sl], zd[:])
            nc.vector.tensor_copy(hbf[:, sl], h_next[:, sl])

    def emit_step(t_idx, j):
        h_prev = h_t[j % 2]
        h_next = h_t[(j + 1) % 2]
        step(t_idx, h_prev, h_next)
        nc.sync.dma_start(
            out_hT[ds(t_idx, 1), :, :].rearrange("t (m p) b -> t p m b", p=128),
            h_next[:].rearrange("p (m b) -> p m b", b=BL))

    if loop_steps >= unroll:
        with tc.For_i(0, loop_steps - loop_steps % unroll, unroll,
                      hint_engines=(mybir.EngineType.PE,)) as iv:
            for j in range(unroll):
                emit_step(iv + j, j)
    for j in range(loop_steps % unroll):
        emit_step(loop_steps - loop_steps % unroll + j, j)


def build(loop_steps=S, unroll=UNROLL):
    nc = bacc.Bacc("TRN2", target_bir_lowering=False, debug=False,
                   num_devices=NCORES)
    io = {
        "xT": nc.dram_tensor("xT", [I, S, BL], F32R, kind="ExternalInput").ap(),
        "h0T": nc.dram_tensor("h0T", [H, BL], F32, kind="ExternalInput").ap(),
        "WT": nc.dram_tensor("WT", [3, I, H], F32R, kind="ExternalInput").ap(),
        "UT": nc.dram_tensor("UT", [3, H, H], BF16, kind="ExternalInput").ap(),
        "bias": nc.dram_tensor("bias", [3, H], F32, kind="ExternalInput").ap(),
        "out_hT": nc.dram_tensor("out_hT", [S, H, BL], F32,
                                 kind="ExternalOutput").ap(),
    }
    with tile.TileContext(nc) as tc:
        with ExitStack() as ctx:
            _emit(ctx, tc, io, loop_steps, unroll)
    nc.compile()
    return nc


def make_in_maps(x, h_0, Wz, bz, Uz, Wr, br, Ur, Wh, bh, Uh):
    x = np.ascontiguousarray(np.asarray(x, dtype=np.float32))
    h_0 = np.asarray(h_0, dtype=np.float32)
    WT = np.ascontiguousarray(
        np.stack([np.asarray(w, np.float32).T for w in (Wz, Wr, Wh)]))
    UT = np.ascontiguousarray(
        np.stack([np.asarray(u, np.float32).T for u in (Uz, Ur, Uh)])
    ).astype(ml_dtypes.bfloat16)
    bias = np.ascontiguousarray(
        np.stack([np.asarray(b, np.float32) for b in (bz, br, bh)]))
    in_maps = []
    for c in range(NCORES):
        xT_c = np.ascontiguousarray(
            x[c * BL:(c + 1) * BL].transpose(2, 1, 0))          # [I, S, BL]
        h0T_c = np.ascontiguousarray(
            np.asarray(h_0, np.float32)[0, c * BL:(c + 1) * BL].T)  # [H, BL]
        in_maps.append({"xT": xT_c, "h0T": h0T_c, "WT": WT, "UT": UT,
                        "bias": bias})
    return in_maps


def assemble(results):
    # per-core out_hT [S, H, BL] -> hidden_seq [B, S, H]
    hidden = np.empty((B, S, H), np.float32)
    for c in range(NCORES):
        # [S, H, BL] -> [BL, S, H]
        hidden[c * BL:(c + 1) * BL] = results[c]["out_hT"].transpose(2, 0, 1)
    h_last = np.ascontiguousarray(hidden[:, -1, :])[None]
    return hidden, h_last


_cached_nc = None


def kernel(x, h_0, Wz, bz, Uz, Wr, br, Ur, Wh, bh, Uh, _trace=False):
    global _cached_nc
    if _cached_nc is None:
        _cached_nc = build()
    in_maps = make_in_maps(x, h_0, Wz, bz, Uz, Wr, br, Ur, Wh, bh, Uh)
    res = run_bass_kernel_spmd(_cached_nc, in_maps,
                               core_ids=list(range(NCORES)), trace=_trace)
    out = assemble(res.results)
    if _trace:
        return out, res
    return out


# revision 14
# speedup vs baseline: 1.4962x; 1.0900x over previous
"""Trainium2 Bass kernel for a custom GRU (B=64, S=512, I=512, H=1024).

Strategy (data-parallel, per the sharding hint):
  - Shard the batch (64) across 8 NeuronCores (8 per core); replicate weights.
  - Phase 1 (per core): x-projections xg = x @ Wg.T + bg for the 3 gates as
    f32r (TF32-like) GEMMs at full PE rate, written to a DRAM scratch buffer
    in a step-major interleaved layout.
  - Phase 2: the sequential recurrence, 512 steps.  Everything lives in a
    transposed "interleaved" layout [128 partitions, 8*m + b] so that all
    elementwise work runs on full 128-partition tiles and the matmul rhs
    slices are contiguous.  U matrices are bf16 (stationary operands, FWL),
    accumulation in fp32 PSUM, gates/state fp32.

Layouts:
  - h (state):  SBUF [128, 64] fp32, element (p, 8*m+b) = h[b, 128*m+p].
  - U tiles:    lhsT per (gate, k-chunk): SBUF [128, 1024] bf16 where
                cols m*128..m*128+128 give the [K=128, M=128] stationary tile.
  - x-proj:     DRAM [512, 128, 192] fp32, (t, p, g*64+8*m+b).
  - out:        DRAM [512, 1024, 8] fp32 (t, h, b_local) per core; assembled
                host-side into [64, 512, 1024] + h_last [1, 64, 1024].
"""

import numpy as np
import ml_dtypes
from contextlib import ExitStack

import concourse.bass as bass
import concourse.mybir as mybir
import concourse.tile as tile
from concourse import bacc
from concourse.bass import ds
from concourse import tile_rust
from concourse.bass_utils import run_bass_kernel_spmd

B, S, I, H = 64, 512, 512, 1024
NCORES = 8
BL = B // NCORES          # 8 batch per core
MT = H // 128             # 8 m-tiles
KC = H // 128             # 8 recurrence k-chunks
PKC = I // 128            # 4 projection k-chunks
NB = (S * BL) // 512      # 8 projection n-blocks (64 steps each)
UNROLL = 32

F32 = mybir.dt.float32
F32R = mybir.dt.float32r
BF16 = mybir.dt.bfloat16
SIG = mybir.ActivationFunctionType.Sigmoid
TANH = mybir.ActivationFunctionType.Tanh


def _emit(ctx, tc, io, loop_steps, unroll):
    nc = tc.nc
    xT, h0T, WT, UT, bias, out_hT = (
        io["xT"], io["h0T"], io["WT"], io["UT"], io["bias"], io["out_hT"])

    const_pool = ctx.enter_context(tc.tile_pool(name="const", bufs=1))
    xt_pool = ctx.enter_context(tc.tile_pool(name="xt", bufs=1))
    w_pool = ctx.enter_context(tc.tile_pool(name="w", bufs=1))
    u_pool = ctx.enter_context(tc.tile_pool(name="u", bufs=1))
    state_pool = ctx.enter_context(tc.tile_pool(name="state", bufs=1))
    stage_pool = ctx.enter_context(tc.tile_pool(name="stage", bufs=3))
    psum_pool = ctx.enter_context(tc.tile_pool(name="psum", bufs=2, space="PSUM"))
    xz_pool = ctx.enter_context(tc.tile_pool(name="xz", bufs=3))
    ew_pool = ctx.enter_context(tc.tile_pool(name="ew", bufs=2))
    dram_pool = ctx.enter_context(tc.tile_pool(name="dram", bufs=1, space="DRAM"))

    # DRAM scratch for the projected inputs: [gate, m, p, t, b] so that the
    # projection evacuation writes 2KB-contiguous per-partition runs and the
    # per-step read is 3 small strided DMAs.
    xp = dram_pool.tile([3, MT, 128, S, BL], F32)

    # ---- load constants -------------------------------------------------
    xt_tiles = []
    for k in range(PKC):
        t = xt_pool.tile([128, S * BL], F32R, tag=f"xt{k}", name=f"xt{k}")
        nc.sync.dma_start(t[:].rearrange("p (s b) -> p s b", b=BL),
                          xT[k * 128:(k + 1) * 128, :, :])
        xt_tiles.append(t)

    w_tiles = {}
    for g in range(3):
        for k in range(PKC):
            t = w_pool.tile([128, H], F32R, tag=f"w{g}_{k}", name=f"w{g}_{k}")
            nc.sync.dma_start(t[:], WT[g, k * 128:(k + 1) * 128, :])
            w_tiles[(g, k)] = t

    u_tiles = {}
    for g in range(3):
        for k in range(KC):
            t = u_pool.tile([128, H], BF16, tag=f"u{g}_{k}", name=f"u{g}_{k}")
            nc.sync.dma_start(t[:], UT[g, k * 128:(k + 1) * 128, :])
            u_tiles[(g, k)] = t

    bias_sb = []
    for g in range(3):
        t = const_pool.tile([128, MT], F32, tag=f"b{g}", name=f"bias{g}")
        nc.sync.dma_start(t[:].rearrange("p m -> p m"),
                          bias[g, :].rearrange("(m p) -> p m", p=128))
        bias_sb.append(t)

    PTAGS = ["p_ra", "p_rb", "p_hlo", "p_hhi"]

    # ---- phase 1: projections ------------------------------------------
    for g in range(3):
        for m in range(MT):
            bsl = bias_sb[g][:, m:m + 1]
            for n in range(NB):
                ps = psum_pool.tile([128, 512], F32, tag=PTAGS[n % 2],
                                    name=f"pp{g}_{m}_{n}")
                for k in range(PKC):
                    nc.tensor.matmul(
                        ps[:],
                        w_tiles[(g, k)][:, m * 128:(m + 1) * 128],
                        xt_tiles[k][:, n * 512:(n + 1) * 512],
                        start=(k == 0), stop=(k == PKC - 1))
                st = stage_pool.tile([128, 512], F32, tag="pstage",
                                     name=f"st{g}_{m}_{n}")
                nc.vector.tensor_scalar_add(st[:], ps[:], bsl)
                nc.sync.dma_start(
                    xp[g, m, :, n * 64:(n + 1) * 64, :],
                    st[:].rearrange("p (s b) -> p s b", b=BL))

    # ---- phase boundary: keep the loop's SBUF reuse ordered after the
    # projection phase's last reads.
    tc.strict_bb_all_engine_barrier()

    # ---- phase 2: recurrence -------------------------------------------
    h_t = [state_pool.tile([128, MT * BL], F32, tag=f"h{i}", name=f"h{i}")
           for i in range(2)]
    hbf = state_pool.tile([128, MT * BL], BF16, tag="hbf", name="hbf")
    nc.sync.dma_start(h_t[0][:].rearrange("p (m b) -> p m b", b=BL),
                      h0T.rearrange("(m p) b -> p m b", p=128))
    nc.vector.tensor_copy(hbf[:], h_t[0][:])

    def step(t_idx, h_prev, h_next):
        # per-step inputs: 3 strided DMAs (one per gate) into [128, 192]
        xz_t = xz_pool.tile([128, 192], F32, tag="xz", name="xz")
        for g in range(3):
            nc.sync.dma_start(
                xz_t[:, g * 64:(g + 1) * 64],
                xp[g, :, :, ds(t_idx, 1), :].rearrange("m p t b -> p m (t b)"))

        # r-gate matmuls first, k-split across two banks so the first half
        # only depends on the low half of hbf (overlaps the previous step's
        # high-half tail).
        psum_ra = psum_pool.tile([128, 64], F32, tag="p_ra", name="psum_ra")
        psum_rb = psum_pool.tile([128, 64], F32, tag="p_rb", name="psum_rb")
        for half, prr in ((0, psum_ra), (1, psum_rb)):
            for m in range(MT):
                for k in range(4 * half, 4 * half + 4):
                    nc.tensor.matmul(
                        prr[:, 8 * m:8 * m + 8],
                        u_tiles[(1, k)][:, m * 128:(m + 1) * 128],
                        hbf[:, 8 * k:8 * k + 8],
                        start=(k == 4 * half), stop=(k == 4 * half + 3))
        tmp_r = ew_pool.tile([128, 64], F32, tag="tmp_r", name="tmp_r")
        nc.vector.tensor_add(tmp_r[:], psum_ra[:], xz_t[:, 64:128])
        tmp_r2 = ew_pool.tile([128, 64], F32, tag="tmp_r2", name="tmp_r2")
        nc.vector.tensor_add(tmp_r2[:], tmp_r[:], psum_rb[:])
        s_r = ew_pool.tile([128, 64], F32, tag="s_r", name="s_r")
        nc.scalar.activation(s_r[:], tmp_r2[:], SIG)
        rh_bf = ew_pool.tile([128, 64], BF16, tag="rh", name="rh_bf")
        rh_ins = nc.vector.tensor_mul(rh_bf[:], s_r[:], h_prev[:])

        psum_z = psum_pool.tile([128, 64], F32, tag="p_z", name="psum_z",
                                bufs=1)
        for m in range(MT):
            for k in range(KC):
                nc.tensor.matmul(
                    psum_z[:, 8 * m:8 * m + 8],
                    u_tiles[(0, k)][:, m * 128:(m + 1) * 128],
                    hbf[:, 8 * k:8 * k + 8],
                    start=(k == 0), stop=(k == KC - 1))

        # h-hat matmuls, split in half (two banks) so the lo tail overlaps
        # the hi matmuls.
        psum_h = [psum_pool.tile([128, 32], F32, tag=PTAGS[2 + hh],
                                 name=f"psum_h{hh}", bufs=1) for hh in range(2)]
        for hh in range(2):
            for m in range(4 * hh, 4 * hh + 4):
                for k in range(KC):
                    nc.tensor.matmul(
                        psum_h[hh][:, 8 * (m - 4 * hh): 8 * (m - 4 * hh) + 8],
                        u_tiles[(2, k)][:, m * 128:(m + 1) * 128],
                        rh_bf[:, 8 * k:8 * k + 8],
                        start=(k == 0), stop=(k == KC - 1))
        # z's elementwise chain is only needed by the tail; emit it after the
        # h-hat matmuls so the DVE FIFO doesn't delay rh behind the z PSUM.
        tmp_z = ew_pool.tile([128, 64], F32, tag="tmp_z", name="tmp_z")
        tz_ins = nc.vector.tensor_add(tmp_z[:], psum_z[:], xz_t[:, 0:64])
        tile_rust.add_dep_helper(tz_ins.ins, rh_ins.ins, sync=False,
                                 reason="keep rh ahead of z chain on DVE")
        s_z = ew_pool.tile([128, 64], F32, tag="s_z", name="s_z")
        nc.scalar.activation(s_z[:], tmp_z[:], SIG)
        # precompute (1-z)*h = h - z*h off the critical tail
        zh = ew_pool.tile([128, 64], F32, tag="zh", name="zh")
        nc.vector.tensor_mul(zh[:], s_z[:], h_prev[:])
        hmu = ew_pool.tile([128, 64], F32, tag="hmu", name="hmu")
        nc.vector.tensor_sub(hmu[:], h_prev[:], zh[:])
        for hh in range(2):
            sl = slice(32 * hh, 32 * hh + 32)
            tmp_h = ew_pool.tile([128, 32], F32, tag=f"tmp_h{hh}",
                                 name=f"tmp_h{hh}")
            nc.vector.tensor_add(tmp_h[:], psum_h[hh][:],
                                 xz_t[:, 128 + 32 * hh:160 + 32 * hh])
            hh_s = ew_pool.tile([128, 32], F32, tag=f"hh_s{hh}",
                                name=f"hh_s{hh}")
            nc.scalar.activation(hh_s[:], tmp_h[:], TANH)
            zd = ew_pool.tile([128, 32], F32, tag=f"zd{hh}", name=f"zd{hh}")
            nc.vector.tensor_mul(zd[:], s_z[:, sl], hh_s[:])
            nc.vector.tensor_add(h_next[:, sl], hmu[:, sl], zd[:])
            nc.vector.tensor_copy(hbf[:, sl], h_next[:, sl])

    def emit_step(t_idx, j):
        h_prev = h_t[j % 2]
        h_next = h_t[(j + 1) % 2]
        step(t_idx, h_prev, h_next)
        nc.sync.dma_start(
            out_hT[ds(t_idx, 1), :, :].rearrange("t (m p) b -> t p m b", p=128),
            h_next[:].rearrange("p (m b) -> p m b", b=BL))

    if loop_steps >= unroll:
        with tc.For_i(0, loop_steps - loop_steps % unroll, unroll,
                      hint_engines=(mybir.EngineType.PE,)) as iv:
            for j in range(unroll):
                emit_step(iv + j, j)
    for j in range(loop_steps % unroll):
        emit_step(loop_steps - loop_steps % unroll + j, j)


def build(loop_steps=S, unroll=UNROLL):
    nc = bacc.Bacc("TRN2", target_bir_lowering=False, debug=False,
                   num_devices=NCORES)
    io = {
        "xT": nc.dram_tensor("xT", [I, S, BL], F32R, kind="ExternalInput").ap(),
        "h0T": nc.dram_tensor("h0T", [H, BL], F32, kind="ExternalInput").ap(),
        "WT": nc.dram_tensor("WT", [3, I, H], F32R, kind="ExternalInput").ap(),
        "UT": nc.dram_tensor("UT", [3, H, H], BF16, kind="ExternalInput").ap(),
        "bias": nc.dram_tensor("bias", [3, H], F32, kind="ExternalInput").ap(),
        "out_hT": nc.dram_tensor("out_hT", [S, H, BL], F32,
                                 kind="ExternalOutput").ap(),
    }
    with tile.TileContext(nc) as tc:
        with ExitStack() as ctx:
            _emit(ctx, tc, io, loop_steps, unroll)
    nc.compile()
    return nc


def make_in_maps(x, h_0, Wz, bz, Uz, Wr, br, Ur, Wh, bh, Uh):
    x = np.ascontiguousarray(np.asarray(x, dtype=np.float32))
    h_0 = np.asarray(h_0, dtype=np.float32)
    WT = np.ascontiguousarray(
        np.stack([np.asarray(w, np.float32).T for w in (Wz, Wr, Wh)]))
    UT = np.ascontiguousarray(
        np.stack([np.asarray(u, np.float32).T for u in (Uz, Ur, Uh)])
    ).astype(ml_dtypes.bfloat16)
    bias = np.ascontiguousarray(
        np.stack([np.asarray(b, np.float32) for b in (bz, br, bh)]))
    in_maps = []
    for c in range(NCORES):
        xT_c = np.ascontiguousarray(
            x[c * BL:(c + 1) * BL].transpose(2, 1, 0))          # [I, S, BL]
        h0T_c = np.ascontiguousarray(
            np.asarray(h_0, np.float32)[0, c * BL:(c + 1) * BL].T)  # [H, BL]
        in_maps.append({"xT": xT_c, "h0T": h0T_c, "WT": WT, "UT": UT,
                        "bias": bias})
    return in_maps


def assemble(results):
    # per-core out_hT [S, H, BL] -> hidden_seq [B, S, H]
    hidden = np.empty((B, S, H), np.float32)
    for c in range(NCORES):
        # [S, H, BL] -> [BL, S, H]
        hidden[c * BL:(c + 1) * BL] = results[c]["out_hT"].transpose(2, 0, 1)
    h_last = np.ascontiguousarray(hidden[:, -1, :])[None]
    return hidden, h_last


_cached_nc = None


def kernel(x, h_0, Wz, bz, Uz, Wr, br, Ur, Wh, bh, Uh, _trace=False):
    global _cached_nc
    if _cached_nc is None:
        _cached_nc = build()
    in_maps = make_in_maps(x, h_0, Wz, bz, Uz, Wr, br, Ur, Wh, bh, Uh)
    res = run_bass_kernel_spmd(_cached_nc, in_maps,
                               core_ids=list(range(NCORES)), trace=_trace)
    out = assemble(res.results)
    if _trace:
        return out, res
    return out
